# revision 54
# baseline (speedup 1.0000x reference)
"""Trainium2 Bass kernel for nn_ActorCritic (soft decision tree actor-critic).

Data-parallel across 8 NeuronCores: each core handles 2048 of 16384 rows.
Per-core compute is laid out "transposed" (feature/node dims on SBUF
partitions, batch rows along the free dim) so that:
  - the node bias folds into the sigmoid as a per-partition ACT bias,
  - the leaf matmul needs no on-device transpose of prob,
  - matmul N (moving dim) is 512 so fp32r runs at full PE rate.

Tree node ordering is position-encoded (level-major, within a level ordered
by the path bits LSB-first) so every doubling step is a contiguous block
operation.  All permutations (heap order <-> position order, final leaf
bit-reversal) are applied host-side.

Hard path: one-hot doubling over exact {0,1} bits taken from the f32 PSUM
pre-activations (DVE is_ge for shallow/mid levels, ACT Sign + DVE remap for
level 9), with the leaf-index extraction and the critic v-dot fused into the
leaf-matmul phase as col-tiled M=1 matmuls.
"""

import sys
import functools

for _p in ("/opt/trn_rl_repo",):
    if _p not in sys.path:
        sys.path.insert(0, _p)

import numpy as np
import ml_dtypes

B = 16384
IN_DIM = 256
OUT_DIM = 64
DEPTH = 10
N_INT = 1023
N_LEAVES = 1024
HID = 512
NCORES = 8
BL = B // NCORES          # 2048 rows per core
S = 512                   # matmul moving-dim slice
NS = BL // S              # 4 slices

BF16 = ml_dtypes.bfloat16


def _bitrev(j, bits):
    r = 0
    for _ in range(bits):
        r = (r << 1) | (j & 1)
        j >>= 1
    return r


@functools.lru_cache(maxsize=1)
def _tables():
    # chunk/partition slot -> heap node index (or -1 for pad)
    # chunk 0: levels 0..6 (slot = 2^d-1+pos), slot 127 pad
    # chunk 1: level 7 pos 0..127; chunks 2-3: level 8; chunks 4-7: level 9
    slot2node = np.full(1024, -1, np.int64)
    for d in range(7):
        base = (1 << d) - 1
        for pos in range(1 << d):
            slot2node[base + pos] = base + _bitrev(pos, d)
    for d, c0 in ((7, 1), (8, 2), (9, 4)):
        base = (1 << d) - 1
        for pos in range(1 << d):
            slot2node[c0 * 128 + pos] = base + _bitrev(pos, d)
    brev10 = np.array([_bitrev(p, 10) for p in range(1024)], np.int64)
    return slot2node, brev10


@functools.lru_cache(maxsize=1)
def _graph():
    import concourse.tile as tile
    from concourse import bacc, mybir
    from contextlib import ExitStack

    f32 = mybir.dt.float32
    f32r = mybir.dt.float32r
    bf16 = mybir.dt.bfloat16
    AF = mybir.ActivationFunctionType
    OP = mybir.AluOpType

    nc = bacc.Bacc()

    xT_d = nc.declare_dram_parameter("xT", [IN_DIM, BL], f32r, isOutput=False)
    Wt_d = nc.declare_dram_parameter("Wt", [IN_DIM, 1024], f32r, isOutput=False)
    bias_d = nc.declare_dram_parameter("bias", [128, 8], f32, isOutput=False)
    bneg_d = nc.declare_dram_parameter("bneg", [128, 8], f32, isOutput=False)
    cw1_d = nc.declare_dram_parameter("cw1", [IN_DIM, HID], f32r, isOutput=False)
    cb1_d = nc.declare_dram_parameter("cb1", [128, 4], f32, isOutput=False)
    cw2_d = nc.declare_dram_parameter("cw2", [128, 4], bf16, isOutput=False)
    leafM_d = nc.declare_dram_parameter("leafM", [1024, OUT_DIM], bf16, isOutput=False)
    idxM_d = nc.declare_dram_parameter("idxM", [128, 8], bf16, isOutput=False)
    identB_d = nc.declare_dram_parameter("identB", [128, 128], bf16, isOutput=False)

    pT_d = nc.declare_dram_parameter("pT", [OUT_DIM, BL], f32, isOutput=True)
    pos_d = nc.declare_dram_parameter("posr", [1, BL], f32, isOutput=True)
    v_d = nc.declare_dram_parameter("vr", [1, BL], f32, isOutput=True)

    with tile.TileContext(nc) as tc:
        with ExitStack() as ctx:
            cp = ctx.enter_context(tc.tile_pool(name="const", bufs=1))
            wp = ctx.enter_context(tc.tile_pool(name="work", bufs=1))
            pp = ctx.enter_context(tc.tile_pool(name="ps", bufs=3, space="PSUM"))
            dp = ctx.enter_context(tc.tile_pool(name="deep", bufs=2))

            # ---- constant loads -------------------------------------------
            cw1 = []
            for k in range(2):
                t = cp.tile([128, HID], f32r, tag=f"cw1{k}")
                nc.scalar.dma_start(t[:], cw1_d[k * 128:(k + 1) * 128, :])
                cw1.append(t)
            bias_sb = cp.tile([128, 8], f32, tag="bias")
            bneg_sb = cp.tile([128, 8], f32, tag="bneg")
            cb1_sb = cp.tile([128, 4], f32, tag="cb1")
            nc.gpsimd.dma_start(cb1_sb[:], cb1_d[:, :])
            cw2_sb = cp.tile([128, 4], bf16, tag="cw2")
            nc.gpsimd.dma_start(cw2_sb[:], cw2_d[:, :])
            leafM = []
            for c in range(8):
                t = cp.tile([128, OUT_DIM], bf16, tag=f"lM{c}")
                nc.gpsimd.dma_start(t[:], leafM_d[c * 128:(c + 1) * 128, :])
                leafM.append(t)
            idx_sb = cp.tile([128, 8], bf16, tag="idxM")
            nc.gpsimd.dma_start(idx_sb[:], idxM_d[:, :])
            identB = cp.tile([128, 128], bf16, tag="identB")
            nc.scalar.dma_start(identB[:], identB_d[:, :])

            xT = []
            wt = []
            for k in range(2):
                t = cp.tile([128, BL], f32r, tag=f"xT{k}")
                xT.append(t)
                t = cp.tile([128, 1024], f32r, tag=f"wt{k}")
                wt.append(t)
            # bias via the ACT queue so the SP queue starts on weights
            nc.scalar.dma_start(bias_sb[:], bias_d[:, :])
            nc.scalar.dma_start(bneg_sb[:], bneg_d[:, :])
            for k in range(2):
                nc.sync.dma_start(wt[k][:, 0:128], Wt_d[k * 128:(k + 1) * 128, 0:128])
            for s in range(NS):
                for k in range(2):
                    nc.sync.dma_start(xT[k][:, s * S:(s + 1) * S],
                                      xT_d[k * 128:(k + 1) * 128, s * S:(s + 1) * S])
            for c in range(1, 8):
                for k in range(2):
                    nc.sync.dma_start(wt[k][:, c * 128:(c + 1) * 128],
                                      Wt_d[k * 128:(k + 1) * 128, c * 128:(c + 1) * 128])

            # ---- val GEMM (transposed), sigmoid, hard bits ----------------
            HF = BL // 2
            valT = {}
            hT = {}
            for c in range(8):
                vt = wp.tile([128, BL], bf16, tag=f"val{c}")
                ht = wp.tile([128, BL], bf16, tag=f"h{c}")
                for hf in range(2):
                    hsl = slice(hf * HF, (hf + 1) * HF)
                    ps = pp.tile([128, HF], f32, tag="ps")
                    for s in range(HF // S):
                        sl = slice(s * S, (s + 1) * S)
                        xsl = slice(hf * HF + s * S, hf * HF + (s + 1) * S)
                        nc.tensor.matmul(ps[:, sl],
                                         wt[0][:, c * 128:(c + 1) * 128],
                                         xT[0][:, xsl], start=True, stop=False)
                        nc.tensor.matmul(ps[:, sl],
                                         wt[1][:, c * 128:(c + 1) * 128],
                                         xT[1][:, xsl], start=False, stop=True)
                    nc.scalar.activation(vt[:, hsl], ps[:], AF.Sigmoid,
                                         bias=bias_sb[:, c:c + 1])
                    if c == 0:
                        # levels 0-6 decide the high leaf bits: exact compare
                        nc.vector.tensor_scalar(ht[:, hsl], ps[:],
                                                bneg_sb[:, c:c + 1],
                                                None, op0=OP.is_ge)
                    else:
                        # deep-level flips only touch low leaf bits; bf16 ok
                        nc.vector.tensor_scalar(ht[:, hsl], vt[:, hsl], 0.5,
                                                None, op0=OP.is_ge)
                valT[c] = vt
                hT[c] = ht

            # ---- shallow levels (per half): transpose chunk0 to row layout ----
            HBv = BL // 2

            def transpose_half(src_ap, dst, hf, via_act):
                psx = pp.tile([128, HBv], bf16, tag="pst", name="psx", bufs=2)
                for t in range(8):
                    col = hf * HBv + t * 128
                    nc.tensor.transpose(psx[:, t * 128:(t + 1) * 128],
                                        src_ap[:, col:col + 128], identB[:])
                if via_act:
                    nc.scalar.copy(dst[:, hf * HBv:(hf + 1) * HBv], psx[:])
                else:
                    nc.vector.tensor_copy(dst[:, hf * HBv:(hf + 1) * HBv], psx[:])

            valsh = wp.tile([128, BL], bf16, tag="valsh")
            hsh = wp.tile([128, BL], bf16, tag="hsh")
            for hf in range(2):
                transpose_half(valT[0][:], valsh, hf, via_act=False)
                transpose_half(hT[0][:], hsh, hf, via_act=True)
            wsh = wp.tile([128, BL], bf16, tag="wsh")
            prA = wp.tile([128, BL], bf16, tag="prA")
            prB = wp.tile([128, BL], bf16, tag="prB")
            ohsh = wp.tile([128, BL], bf16, tag="h0")
            p7T = wp.tile([128, BL], bf16, tag="p7Tt")
            o7T = wp.tile([128, BL], bf16, tag="o7Tt")
            for hf in range(2):
                hsl = slice(hf * HBv, (hf + 1) * HBv)
                # w = 1 - v so left/right children need no serial subtract
                nc.vector.tensor_scalar(wsh[:, hsl], valsh[:, hsl], -1.0, 1.0,
                                        op0=OP.mult, op1=OP.add)
                vs = valsh[:, hsl].rearrange("p (t s) -> p t s", s=128)
                ws = wsh[:, hsl].rearrange("p (t s) -> p t s", s=128)
                gs = hsh[:, hsl].rearrange("p (t s) -> p t s", s=128)
                pa = prA[:, hsl].rearrange("p (t s) -> p t s", s=128)
                pb = prB[:, hsl].rearrange("p (t s) -> p t s", s=128)
                os_ = ohsh[:, hsl].rearrange("p (t s) -> p t s", s=128)
                nc.vector.memset(pa[:, :, 0:1], 1.0)
                nc.vector.memset(os_[:, :, 0:1], 1.0)
                cur, nxt = pa, pb
                for d in range(7):
                    n = 1 << d
                    base = n - 1
                    nc.vector.tensor_mul(nxt[:, :, n:2 * n], cur[:, :, 0:n],
                                         vs[:, :, base:base + n])
                    nc.vector.tensor_mul(nxt[:, :, 0:n], cur[:, :, 0:n],
                                         ws[:, :, base:base + n])
                    cur, nxt = nxt, cur
                    # hard one-hot: right then left (exact 0/1 products)
                    nc.vector.tensor_mul(os_[:, :, n:2 * n], os_[:, :, 0:n],
                                         gs[:, :, base:base + n])
                    nc.vector.tensor_sub(os_[:, :, 0:n], os_[:, :, 0:n],
                                         os_[:, :, n:2 * n])
                # after 7 levels prob7 sits in prB
                transpose_half(prB[:], p7T, hf, via_act=False)
                transpose_half(ohsh[:], o7T, hf, via_act=True)

            # ---- critic GEMM + relu (v-dot deferred to leaf phase) --------
            hcrit = []
            for c in range(4):
                hc = wp.tile([128, BL], bf16, tag=f"hcrit{c}")
                for hf in range(2):
                    hsl = slice(hf * HF, (hf + 1) * HF)
                    ps = pp.tile([128, HF], f32, tag="ps")
                    for s in range(HF // S):
                        sl = slice(s * S, (s + 1) * S)
                        xsl = slice(hf * HF + s * S, hf * HF + (s + 1) * S)
                        nc.tensor.matmul(ps[:, sl],
                                         cw1[0][:, c * 128:(c + 1) * 128],
                                         xT[0][:, xsl], start=True, stop=False)
                        nc.tensor.matmul(ps[:, sl],
                                         cw1[1][:, c * 128:(c + 1) * 128],
                                         xT[1][:, xsl], start=False, stop=True)
                    nc.scalar.activation(hc[:, hsl], ps[:], AF.Relu,
                                         bias=cb1_sb[:, c:c + 1])
                hcrit.append(hc)

            # negated deep gates for chunks 1-3 (parallel left/right muls)
            wneg = {}
            for c in (1, 2, 3):
                t = wp.tile([128, BL], bf16, tag=["prA", "prB", "wsh"][c - 1], name="wnegt")
                for hf in range(2):
                    hsl = slice(hf * (BL // 2), (hf + 1) * (BL // 2))
                    nc.vector.tensor_scalar(t[:, hsl], valT[c][:, hsl], -1.0,
                                            1.0, op0=OP.mult, op1=OP.add)
                wneg[c] = t

            # ---- deep doubling + leaf/extract/vdot, pipelined in halves ----
            HB = BL // 2
            stage = wp.tile([128, BL], f32, tag="val0")
            for half in range(2):
                hs = slice(half * HB, (half + 1) * HB)

                def mk(tag):
                    return dp.tile([128, HB], bf16, tag=tag, name=tag)

                def mul(tag, a, b):
                    t = mk(tag)
                    nc.vector.tensor_mul(t[:], a, b)
                    return t

                def sub(tag, a, b):
                    t = mk(tag)
                    nc.vector.tensor_sub(t[:], a, b)
                    return t

                p8r = mul("p8r", p7T[:, hs], valT[1][:, hs])
                p8l = mul("p8l", p7T[:, hs], wneg[1][:, hs])
                o8r = mul("o8r", o7T[:, hs], hT[1][:, hs])
                o8l = sub("o8l", o7T[:, hs], o8r[:])
                p9 = [None] * 4
                o9 = [None] * 4
                p9[2] = mul("p9c2", p8l[:], valT[2][:, hs])
                p9[0] = mul("p9c0", p8l[:], wneg[2][:, hs])
                p9[3] = mul("p9c3", p8r[:], valT[3][:, hs])
                p9[1] = mul("p9c1", p8r[:], wneg[3][:, hs])
                o9[2] = mul("o9c2", o8l[:], hT[2][:, hs])
                o9[0] = sub("o9c0", o8l[:], o9[2][:])
                o9[3] = mul("o9c3", o8r[:], hT[3][:, hs])
                o9[1] = sub("o9c1", o8r[:], o9[3][:])
                pvtags = ["p8r", "p8l", "pv9c2", "pv9c3"]
                ohtags = ["o8r", "o8l", "oh9c2", "oh9c3"]
                pv9 = []
                for c in range(4):
                    t = dp.tile([128, HB], bf16, tag=pvtags[c], name="pvt")
                    eng = nc.gpsimd if c < 3 else nc.vector
                    eng.tensor_mul(t[:], p9[c][:], valT[4 + c][:, hs])
                    pv9.append(t)
                oh9 = []
                for c in range(4):
                    t = dp.tile([128, HB], bf16, tag=ohtags[c], name="oht")
                    eng = nc.gpsimd if c < 2 else nc.vector
                    eng.tensor_mul(t[:], o9[c][:], hT[4 + c][:, hs])
                    oh9.append(t)
                probstack = p9 + pv9
                ohstack = o9 + oh9
                ps_pi = pp.tile([128, HB], f32, tag="ps")
                corder = list(range(8))
                for ci, c in enumerate(corder):
                    for s in range(HB // S):
                        sl = slice(s * S, (s + 1) * S)
                        gsl = slice(half * HB + s * S, half * HB + (s + 1) * S)
                        nc.tensor.matmul(ps_pi[0:OUT_DIM, sl], leafM[c][:],
                                         probstack[c][:, sl],
                                         start=(ci == 0), stop=(ci == 7),
                                         tile_position=(0, 0))
                        nc.tensor.matmul(ps_pi[64:65, sl], idx_sb[:, c:c + 1],
                                         ohstack[c][:, sl],
                                         start=(ci == 0), stop=(ci == 7),
                                         tile_position=(0, 64))
                        if ci < 4:
                            nc.tensor.matmul(ps_pi[96:97, sl],
                                             cw2_sb[:, ci:ci + 1],
                                             hcrit[ci][:, gsl],
                                             start=(ci == 0), stop=(ci == 3),
                                             tile_position=(0, 96))
                nc.scalar.copy(stage[0:OUT_DIM, hs], ps_pi[0:OUT_DIM, :])
                nc.scalar.copy(stage[64:97, hs], ps_pi[64:97, :])
                nc.sync.dma_start(pT_d[:, hs], stage[0:OUT_DIM, hs])
                nc.sync.dma_start(pos_d[:, hs], stage[64:65, hs])
                nc.sync.dma_start(v_d[:, hs], stage[96:97, hs])

    nc.compile()
    return nc


def _host_inputs(x, W_nodes, b_nodes, leaf_dist, cw1, cb1, cw2):
    slot2node, brev10 = _tables()
    valid = slot2node >= 0
    Wt = np.zeros((IN_DIM, 1024), np.float32)
    Wt[:, valid] = np.asarray(W_nodes, np.float32)[slot2node[valid]].T
    bias = np.zeros(1024, np.float32)
    bias[valid] = np.asarray(b_nodes, np.float32)[slot2node[valid]]
    bias_sb = bias.reshape(8, 128).T.copy()          # [128, 8]
    bneg_sb = (-bias).reshape(8, 128).T.copy()

    leafP = np.asarray(leaf_dist, np.float32)[brev10]      # position order
    leafM = np.concatenate([leafP[:512], leafP[512:] - leafP[:512]], axis=0)
    idx = np.concatenate([np.arange(512, dtype=np.float32) - 256.0,
                          np.full(512, 512.0, np.float32)])
    idx_sb = idx.reshape(8, 128).T.copy()

    cw2c = np.asarray(cw2, np.float32).reshape(4, 128).T.copy()
    cb1_sb = np.asarray(cb1, np.float32).reshape(4, 128).T.copy()

    common = {
        "Wt": Wt,
        "bias": bias_sb,
        "bneg": bneg_sb,
        "cw1": np.asarray(cw1, np.float32),
        "cb1": cb1_sb,
        "cw2": cw2c.astype(BF16),
        "leafM": leafM.astype(BF16),
        "idxM": idx_sb.astype(BF16),
        "identB": np.eye(128, dtype=np.float32).astype(BF16),
    }
    x = np.asarray(x, np.float32)
    in_maps = []
    for i in range(NCORES):
        xs = np.ascontiguousarray(x[i * BL:(i + 1) * BL].T)
        in_maps.append({**common, "xT": xs})
    return in_maps


def _run(in_maps, trace=False):
    from concourse.bass_utils import run_bass_kernel_spmd
    nc = _graph()
    return run_bass_kernel_spmd(nc, in_maps, core_ids=list(range(NCORES)),
                                trace=trace)


def kernel(x, W_nodes, b_nodes, leaf_dist, cw1, cb1, cw2, cb2):
    _, brev10 = _tables()
    in_maps = _host_inputs(x, W_nodes, b_nodes, leaf_dist, cw1, cb1, cw2)
    res = _run(in_maps).results

    p = np.concatenate([np.asarray(r["pT"], np.float32).T for r in res], axis=0)
    pos = np.concatenate([np.asarray(r["posr"], np.float32)[0] for r in res])
    v = np.concatenate([np.asarray(r["vr"], np.float32)[0] for r in res])

    pos10 = np.clip(np.rint(pos + 256.0).astype(np.int64), 0, 1023)
    leaf_idx = brev10[pos10].astype(np.int32)
    v_out = (v + np.float32(np.asarray(cb2, np.float32).reshape(-1)[0]))
    return p, leaf_idx, v_out.reshape(B, 1).astype(np.float32)
